# revision 1
# baseline (speedup 1.0000x reference)
"""Trainium2 Bass kernel for nn_MultiHeadAttention (B=4, S=2048, D=768, H=12).

Sharding: query-parallel. 8 cores = 4 batches x 2 query-halves. Each core
computes full K/V projections for its batch (duplicated across the 2 cores
sharing a batch) plus Q projection / attention / output projection / LayerNorm
for its 1024 query rows. No collectives needed: each core owns a disjoint
slice of the output.

Head pairs (2 heads = one 128-partition chunk) are processed together: the
two 64-row score matmuls run CONCURRENTLY in the PE array via row tiling
(tile_position (0,0) and (64,0) auto-derived from base partitions), so scores
cost ~N cycles per pair instead of 2N. The exp on ACT is the critical path
(~1.15us per [128,1024] tile); masks run on DVE, AV + projections on PE are
scheduled to hide under it. AV matmuls lag the exp/mask pipeline by one
key-chunk so the in-order PE queue never stalls waiting on DVE.

PSUM budget (8 banks):
  sA, sB  2x [128,1024] f32  score tiles (A/B alternate as exp double-buffer)
  cx      4x [128, 512] f32  ctx accum [65,512] per (head, q-half); slots are
                             borrowed between pairs for kproj groups and in
                             phase 1 for vproj/qhat groups.
Rowsum: ones-column in the AV weights -> row 64 of ctx; rows copied to SBUF,
one reciprocal per pair, one DRAM hop broadcasts recips to 64 partitions;
ctxT is copied unnormalized (fast PSUM slot release) and normalized in SBUF
mid-next-pair, off the ACT critical path.
"""

import sys

for _p in ("/opt/trn_rl_repo", "/root/.axon_site/_ro/trn_rl_repo"):
    if _p not in sys.path:
        sys.path.insert(0, _p)

import numpy as np
import ml_dtypes

B = 4
S = 2048
D = 768
H = 12
DK = 64
NCORES = 8
ROWS = S // 2          # 1024 query rows per core
P = 128
KO = D // P            # 6 contraction chunks
MC = D // P            # 6 head-pair chunks
KC = S // P            # 16 key chunks
RC = ROWS // P         # 8 row chunks
VW = DK + 1            # 65: v columns + ones column
EPS = 1e-5
NS = 512               # PSUM bank = 512 f32; matmul out must stay in one bank

BF16 = ml_dtypes.bfloat16

_cached = {}

LDW_OPT = False  # walrus ldw-opt is incompatible with this lowering path


def _enable_ldw_opt():
    if _cached.get("ldw_patched"):
        return
    import concourse.bass_utils as bu

    orig = bu.run_command

    def patched(argv, **kwargs):
        argv = ["--enable-ldw-opt=true" if a == "--enable-ldw-opt=false" else a
                for a in argv]
        return orig(argv, **kwargs)

    bu.run_command = patched
    _cached["ldw_patched"] = True


def _build():
    import concourse.bass as bass
    import concourse.tile as tile
    import concourse.mybir as mybir
    from concourse import bacc

    f32 = mybir.dt.float32
    bf = mybir.dt.bfloat16
    f8 = mybir.dt.float8e4
    AF = mybir.ActivationFunctionType
    OP = mybir.AluOpType
    DR = mybir.MatmulPerfMode.DoubleRow

    if LDW_OPT:
        _enable_ldw_opt()

    nc = bacc.Bacc("TRN2", target_bir_lowering=False, debug=False)

    qt_d = nc.dram_tensor("qt", [D, ROWS], f8, kind="ExternalInput")
    kt_d = nc.dram_tensor("kt", [D, S], f8, kind="ExternalInput")
    vt_d = nc.dram_tensor("vt", [D, S], f8, kind="ExternalInput")
    keep_d = nc.dram_tensor("keep", [S, ROWS], bf, kind="ExternalInput")
    qres_d = nc.dram_tensor("qres", [ROWS, D], bf, kind="ExternalInput")
    w_d = {n: nc.dram_tensor(n, [D, D], f8 if n in ("wq", "wk", "wv") else bf,
                             kind="ExternalInput")
           for n in ("wq", "wk", "wv", "wo")}
    ident_d = nc.dram_tensor("ident", [P, P], bf, kind="ExternalInput")
    b_d = {n: nc.dram_tensor(n, [D], f32, kind="ExternalInput")
           for n in ("bq", "bk", "bv", "gamma", "beta")}
    out_d = nc.dram_tensor("out", [ROWS, D], f32, kind="ExternalOutput")

    # rowsum bounce (per head-pair): f32 rowsums out (reloaded transposed so
    # the reciprocal runs on [128,16]), bf16 recips out, broadcast back
    rs_d = [nc.dram_tensor(f"rs_bounce{mc}", [2 * ROWS], bf, kind="Internal")
            for mc in range(MC)]
    rs2_d = [nc.dram_tensor(f"rs2_bounce{mc}", [2, ROWS], bf, kind="Internal")
             for mc in range(MC)]

    def bcast_ap(handle, n, row=0):
        ap = handle.ap()
        return bass.AP(tensor=ap.tensor, offset=row * n, ap=[[0, DK], [1, n]])

    def bcast_ap_p(handle, n):
        ap = handle.ap()
        return bass.AP(tensor=ap.tensor, offset=0, ap=[[0, P], [1, n]])

    with tile.TileContext(nc) as tc:
        with tc.tile_pool(name="wp", bufs=1) as wp, \
             tc.tile_pool(name="xin", bufs=2) as xin, \
             tc.tile_pool(name="kp", bufs=2) as kp, \
             tc.tile_pool(name="ktp", bufs=2) as ktp, \
             tc.tile_pool(name="big", bufs=1) as big, \
             tc.tile_pool(name="pp", bufs=6) as ppool, \
             tc.tile_pool(name="small", bufs=2) as small, \
             tc.tile_pool(name="ph3", bufs=2) as ph3, \
             tc.tile_pool(name="ps", bufs=1, space="PSUM") as psp:

            def s_tile(tag, name):
                return psp.tile([P, ROWS], f32, tag=tag, name=name)

            # ---- PE warm-up in the DMA shadow: ~4.5us of dummy matmuls so
            # the HAM clock gate reaches 8/8 before the real work arrives ----
            warm = wp.tile([P, P], bf, tag="warm")
            nc.vector.memset(warm, 0.0)
            wps = psp.tile([P, NS], f32, tag="sA", name="warm_ps")
            for i in range(40):
                nc.tensor.matmul(wps[:, 0:P], warm, warm,
                                 start=(i == 0), stop=(i == 39))

            # ---- weights (DMA issue order = arrival order; wq first) ----
            w_sb = {}

            def load_w(n):
                dt = f8 if n in ("wq", "wk", "wv") else bf
                t = wp.tile([P, KO, D], dt, tag=n, name=f"w_{n}")
                src = w_d[n].ap().rearrange("(o p) n -> p o n", p=P)
                for ko in range(KO):
                    nc.sync.dma_start(out=t[:, ko, :], in_=src[:, ko, :])
                w_sb[n] = t

            bq_sb = wp.tile([P, MC], f32, tag="bq")
            bk_sb = wp.tile([P, MC], f32, tag="bk")
            nc.sync.dma_start(out=bq_sb, in_=b_d["bq"].ap().rearrange("(o p) -> p o", p=P))
            nc.sync.dma_start(out=bk_sb, in_=b_d["bk"].ap().rearrange("(o p) -> p o", p=P))
            eps_t = wp.tile([P, 1], f32, tag="eps")
            nc.vector.memset(eps_t, EPS)

            # ---- phase 1a: q~^T inputs (wq first so PE starts ASAP) ----
            wq_t = wp.tile([P, KO, D], f8, tag="wq", name="w_wq")
            wq_src = w_d["wq"].ap().rearrange("(o p) n -> p o n", p=P)
            w_sb["wq"] = wq_t
            qt_sb = xin.tile([P, KO, ROWS], f8, tag="xin", name="qt_sb")
            qt_src = qt_d.ap().rearrange("(o p) r -> p o r", p=P)
            for ko in range(KO):
                nc.sync.dma_start(out=wq_t[:, ko, :], in_=wq_src[:, ko, :])
                nc.sync.dma_start(out=qt_sb[:, ko, :], in_=qt_src[:, ko, :])

            load_w("wk")
            kt_sb = []
            kt_src = kt_d.ap().rearrange("(o p) s -> p o s", p=P)
            for half in range(2):
                t = ktp.tile([P, KO, ROWS], f8, tag="kt", name=f"kt_sb{half}")
                for ko in range(KO):
                    nc.sync.dma_start(
                        out=t[:, ko, :],
                        in_=kt_src[:, ko, half * ROWS:(half + 1) * ROWS])
                kt_sb.append(t)

            # keep mask rides the scalar engine's DMA queue so the sync queue
            # stays free for the input/weight loads the PE is waiting on
            keep_sb = big.tile([P, KC, ROWS], bf, tag="keep")
            keep_src = keep_d.ap().rearrange("(c p) r -> p c r", p=P)
            for c in range(4):
                nc.scalar.dma_start(out=keep_sb[:, c, :], in_=keep_src[:, c, :])

            load_w("wv")
            vt_sb = []
            vt_src = vt_d.ap().rearrange("(o p) s -> p o s", p=P)
            for half in range(2):
                t = xin.tile([P, KO, ROWS], f8, tag="xin", name=f"vt_sb{half}")
                for ko in range(KO):
                    nc.sync.dma_start(
                        out=t[:, ko, :],
                        in_=vt_src[:, ko, half * ROWS:(half + 1) * ROWS])
                vt_sb.append(t)

            bvb = wp.tile([P, D], bf, tag="bvb")
            gb = wp.tile([P, D], bf, tag="gb")
            bb = wp.tile([P, D], bf, tag="bb")
            nc.gpsimd.dma_start(out=bvb, in_=bcast_ap_p(b_d["bv"], D))
            nc.gpsimd.dma_start(out=gb, in_=bcast_ap_p(b_d["gamma"], D))
            nc.gpsimd.dma_start(out=bb, in_=bcast_ap_p(b_d["beta"], D))

            for c in range(4, KC):
                nc.scalar.dma_start(out=keep_sb[:, c, :], in_=keep_src[:, c, :])
            load_w("wo")
            ident_sb = wp.tile([P, P], bf, tag="ident")
            nc.scalar.dma_start(out=ident_sb, in_=ident_d.ap())
            qres_sb = big.tile([P, RC, D], bf, tag="qres")
            qres_src = qres_d.ap().rearrange("(c p) d -> p c d", p=P)
            for rc in range(RC):
                nc.scalar.dma_start(out=qres_sb[:, rc, :], in_=qres_src[:, rc, :])

            # ---- phase 1: qhat (ACT copies: ACT is idle pre-attention) ----
            qhat = big.tile([P, MC, ROWS], bf, tag="qhat")
            for mc in range(MC):
                ps = s_tile("sA" if mc % 2 == 0 else "sB", f"qh_{mc}")
                for kp2 in range(KO // 2):
                    for n0 in (0, 512):
                        nc.tensor.matmul(
                            ps[:, n0:n0 + NS],
                            w_sb["wq"][:, 2 * kp2:2 * kp2 + 2, mc * P:(mc + 1) * P],
                            qt_sb[:, 2 * kp2:2 * kp2 + 2, n0:n0 + NS],
                            start=(kp2 == 0), stop=(kp2 == KO // 2 - 1),
                            perf_mode=DR)
                nc.scalar.activation(out=qhat[:, mc, :], in_=ps, func=AF.Identity,
                                     bias=bq_sb[:, mc:mc + 1], scale=1.0)

            # ---- v~ (keys on partitions); chunks 0-7 up front, 8-15 in pair0
            vhat = big.tile([P, KC, H * VW], bf, tag="vhat")
            nc.vector.memset(
                vhat.rearrange("p c (h w) -> p c h w", w=VW)[:, :, :, DK:DK + 1], 1.0)

            def vproj(kc, tag):
                half, c = divmod(kc, KC // 2)
                ps = psp.tile([P, D], f32, tag=tag, name=f"vp_{kc}")
                for kp2 in range(KO // 2):
                    for n0 in (0, 512):
                        n1 = min(n0 + NS, D)
                        nc.tensor.matmul(
                            ps[:, n0:n1],
                            vt_sb[half][:, 2 * kp2:2 * kp2 + 2, c * P:(c + 1) * P],
                            w_sb["wv"][:, 2 * kp2:2 * kp2 + 2, n0:n1],
                            start=(kp2 == 0), stop=(kp2 == KO // 2 - 1),
                            perf_mode=DR)
                dst = vhat.rearrange("p c (h w) -> p c h w", w=VW)[:, kc, :, 0:DK]
                nc.vector.tensor_tensor(
                    out=dst, in0=ps.rearrange("p (h w) -> p h w", w=DK),
                    in1=bvb.rearrange("p (h w) -> p h w", w=DK), op=OP.add)

            for kc in range(KC):
                vproj(kc, "sA" if kc % 2 == 0 else "sB")

            # ---- kproj: kbuf[mc] = k~^T cols for head pair mc ------------
            kbufs = [None] * MC

            def kproj_group(mc, g, tag="cx", dr=False):
                half, q0 = divmod(g, 2)
                ps = psp.tile([P, NS], f32, tag=tag, bufs=4 if tag == "cx" else 1,
                              name=f"kp_{mc}_{g}")
                if dr:
                    for kp2 in range(KO // 2):
                        nc.tensor.matmul(
                            ps,
                            w_sb["wk"][:, 2 * kp2:2 * kp2 + 2, mc * P:(mc + 1) * P],
                            kt_sb[half][:, 2 * kp2:2 * kp2 + 2,
                                        q0 * NS:(q0 + 1) * NS],
                            start=(kp2 == 0), stop=(kp2 == KO // 2 - 1),
                            perf_mode=DR)
                else:
                    for ko in range(KO):
                        nc.tensor.matmul(
                            ps,
                            w_sb["wk"][:, ko, mc * P:(mc + 1) * P],
                            kt_sb[half][:, ko, q0 * NS:(q0 + 1) * NS],
                            start=(ko == 0), stop=(ko == KO - 1))
                nc.vector.tensor_scalar(
                    out=kbufs[mc][:, g * NS:(g + 1) * NS], in0=ps,
                    scalar1=bk_sb[:, mc:mc + 1], scalar2=None, op0=OP.add)

            kbufs[0] = kp.tile([P, S], bf, tag="kbuf", name="kbuf_0")
            for g in range(4):
                kproj_group(0, g, dr=True)

            # ---- attention: head pairs ----------------------------------
            ctxT = big.tile([P, MC, ROWS], bf, tag="ctxT")
            rsb_tiles = {}
            rsT_tiles = {}
            vh4 = vhat.rearrange("p c (h w) -> p c h w", w=VW)

            def recip_pair(mc):
                # reciprocal on transposed [128,16] rowsums (cheap on DVE),
                # then bounce to DRAM for the 64-partition broadcast
                rsT = rsT_tiles.pop(mc)
                with nc.allow_low_precision(reason="softmax rowsum recip bf16"):
                    nc.vector.reciprocal(out=rsT, in_=rsT)
                nc.sync.dma_start(
                    out=rs2_d[mc].ap().rearrange("a b -> (a b)")
                    .rearrange("(p o) -> p o", p=P), in_=rsT)
                rsb = small.tile([P, ROWS], bf, tag="rsb", name=f"rsb_{mc}")
                rsb_tiles[mc] = rsb
                for hh in range(2):
                    nc.gpsimd.dma_start(
                        out=rsb[hh * DK:(hh + 1) * DK, :],
                        in_=bcast_ap(rs2_d[mc], ROWS, row=hh))

            def finish_pair(mc):
                # normalize ctxT for pair mc (recips already broadcast back)
                nc.vector.tensor_tensor(out=ctxT[:, mc, :], in0=ctxT[:, mc, :],
                                        in1=rsb_tiles.pop(mc), op=OP.mult)

            for mc in range(MC):
                kbuf = kbufs[mc]
                ctx = [[psp.tile([VW, NS], f32, tag="cx", bufs=4,
                                 name=f"ctx_{mc}_{hh}_{qh}")
                        for qh in range(2)] for hh in range(2)]
                pend = None  # (kc, p_A, p_B) awaiting AV

                for kc in range(KC):
                    # deferred rowsum recip / normalize, off critical path
                    if kc == 5 and mc > 0:
                        recip_pair(mc - 1)
                    if kc == 11 and mc > 0:
                        finish_pair(mc - 1)

                    # concurrent row-tiled scores for the two heads
                    s_ps = [s_tile("sA", f"s_{mc}_{kc}_0"),
                            s_tile("sB", f"s_{mc}_{kc}_1")]
                    for n0 in (0, 512):
                        for hh in range(2):
                            pr = slice(hh * DK, (hh + 1) * DK)
                            nc.tensor.matmul(s_ps[hh][:, n0:n0 + NS],
                                             kbuf[pr, kc * P:(kc + 1) * P],
                                             qhat[pr, mc, n0:n0 + NS],
                                             start=True, stop=True)
                    p_ts = []
                    for hh in range(2):
                        p_t = ppool.tile([P, ROWS], bf, tag="p",
                                         name=f"p_{mc}_{kc}_{hh}")
                        nc.scalar.activation(out=p_t, in_=s_ps[hh], func=AF.Exp,
                                             scale=1.0 / np.sqrt(DK))
                        nc.vector.tensor_tensor(out=p_t, in0=p_t,
                                                in1=keep_sb[:, kc, :], op=OP.mult)
                        p_ts.append(p_t)

                    # AV lags one key-chunk so PE never waits on DVE
                    def do_av(kcav, pa, pb):
                        for hh, pt in ((0, pa), (1, pb)):
                            h = 2 * mc + hh
                            for qh in range(2):
                                nc.tensor.matmul(
                                    ctx[hh][qh][:, :],
                                    vh4[:, kcav, h, :],
                                    pt[:, qh * NS:(qh + 1) * NS],
                                    start=(kcav == 0), stop=(kcav == KC - 1))
                    if pend is not None:
                        do_av(*pend)
                    pend = (kc, p_ts[0], p_ts[1])
                do_av(*pend)

                # pair boundary: first kproj group of the next pair rides the
                # freed sB slot; rowsum-row + ctxT copies free the cx slots,
                # each immediately reused by the next kproj group.
                if mc + 1 < MC:
                    kbufs[mc + 1] = kp.tile([P, S], bf, tag="kbuf",
                                            name=f"kbuf_{mc + 1}")
                    kproj_group(mc + 1, 0, tag="sB", dr=True)  # sB frees first
                rs_t = small.tile([DK + 1, 2 * ROWS], bf, tag="rs", bufs=1,
                                  name=f"rs_{mc}")
                for hh in range(2):
                    for qh in range(2):
                        nc.vector.tensor_copy(
                            out=rs_t[DK:DK + 1,
                                     hh * ROWS + qh * NS:hh * ROWS + (qh + 1) * NS],
                            in_=ctx[hh][qh][DK:DK + 1, :])
                # rowsum row -> transposed [128,16] via SBUF-to-SBUF DMA
                rsT = small.tile([P, 2 * ROWS // P], bf, tag="rsT",
                                 name=f"rsT_{mc}")
                rsT_tiles[mc] = rsT
                nc.sync.dma_start(out=rsT, in_=rs_t[DK:DK + 1, :])
                for g, (hh, qh) in enumerate(((0, 0), (0, 1), (1, 0), (1, 1))):
                    pr = slice(hh * DK, (hh + 1) * DK)
                    nc.vector.tensor_copy(
                        out=ctxT[pr, mc, qh * NS:(qh + 1) * NS],
                        in_=ctx[hh][qh][0:DK, :])
                    if mc + 1 < MC and g >= 1:
                        kproj_group(mc + 1, g, dr=True)
            # preload the sqrt table set while the last bounce is in flight
            sq_warm = small.tile([1, 1], f32, tag="sqw")
            nc.scalar.activation(out=sq_warm, in_=eps_t[0:1, 0:1], func=AF.Sqrt,
                                 bias=eps_t[0:1, 0:1], scale=1.0)
            recip_pair(MC - 1)
            finish_pair(MC - 1)

            # ---- phase 3: out projection + residual + LayerNorm ----
            # Contraction order puts the last pair's ctxT chunk (mc=5, whose
            # normalization lands latest) at the end; rc0/rc1's first 5 chunks
            # run under the last normalize chain.
            nsub = 2
            sub = D // nsub  # 384 <= BN_STATS_FMAX (512)
            op_ps = {}

            def oproj_part(rc, kos, last=False):
                if rc not in op_ps:
                    op_ps[rc] = psp.tile([P, D], f32,
                                         tag="sA" if rc % 2 == 0 else "sB",
                                         name=f"op_{rc}")
                for ko in kos:
                    for n0 in (0, 512):
                        n1 = min(n0 + NS, D)
                        nc.tensor.matmul(
                            op_ps[rc][:, n0:n1],
                            ctxT[:, ko, rc * P:(rc + 1) * P],
                            w_sb["wo"][:, ko, n0:n1],
                            start=(ko == 0), stop=False)
                if last:
                    # residual add via identity matmul (PE, not DVE)
                    for n0 in (0, 512):
                        n1 = min(n0 + NS, D)
                        nc.tensor.matmul(op_ps[rc][:, n0:n1], ident_sb,
                                         qres_sb[:, rc, n0:n1],
                                         start=False, stop=True)

            oproj_part(0, range(KO - 1))
            oproj_part(1, range(KO - 1))
            for rc in range(RC):
                if rc < 2:
                    oproj_part(rc, [KO - 1], last=True)
                else:
                    oproj_part(rc, range(KO), last=True)
                ps = op_ps.pop(rc)

                stats = small.tile([P, nsub, 6], f32, tag="stats")
                for sg in range(nsub):
                    nc.vector.bn_stats(out=stats[:, sg, :],
                                       in_=ps[:, sg * sub:(sg + 1) * sub])
                mv = small.tile([P, 2], f32, tag="mv")
                nc.vector.bn_aggr(out=mv, in_=stats)
                std_t = small.tile([P, 1], f32, tag="std")
                nc.scalar.activation(out=std_t, in_=mv[:, 1:2], func=AF.Sqrt,
                                     bias=eps_t, scale=1.0)
                nc.vector.reciprocal(out=std_t, in_=std_t)
                x_t = ph3.tile([P, D], bf, tag="x")
                nc.vector.tensor_scalar(out=x_t, in0=ps, scalar1=mv[:, 0:1],
                                        scalar2=std_t, op0=OP.subtract,
                                        op1=OP.mult)
                xf = ph3.tile([P, D], f32, tag="xf")
                nc.gpsimd.tensor_tensor(out=x_t, in0=x_t, in1=gb, op=OP.mult)
                nc.gpsimd.tensor_tensor(out=xf, in0=x_t, in1=bb, op=OP.add)
                nc.sync.dma_start(out=out_d.ap()[rc * P:(rc + 1) * P, :], in_=xf)

    nc.compile()
    return nc


def _get_nc():
    if "nc" not in _cached:
        _cached["nc"] = _build()
    return _cached["nc"]


def _make_in_maps(Q, Kt, V, attn_mask, Wq, bq, Wk, bk, Wv, bv, Wo, bo, gamma, beta):
    f32 = np.float32
    FP8 = ml_dtypes.float8_e4m3
    # fp8 DoubleRow projections: weights x8 / activations /8 so products are
    # exact; weights land in fp8 normal range, activations keep headroom.
    w = {"wq": (np.ascontiguousarray(Wq, f32) * 8).astype(FP8),
         "wk": (np.ascontiguousarray(Wk, f32) * 8).astype(FP8),
         "wv": (np.ascontiguousarray(Wv, f32) * 8).astype(FP8),
         "wo": np.ascontiguousarray(Wo, f32).astype(BF16)}
    b = {"bq": np.ascontiguousarray(bq, f32), "bk": np.ascontiguousarray(bk, f32),
         "bv": np.ascontiguousarray(bv, f32),
         "gamma": np.ascontiguousarray(gamma, f32),
         "beta": np.ascontiguousarray(beta, f32)}
    bo_f = np.asarray(bo, f32)
    in_maps = []
    for c in range(NCORES):
        bidx, half = divmod(c, 2)
        rows = slice(half * ROWS, (half + 1) * ROWS)
        m = {
            "qt": (np.ascontiguousarray(Q[bidx, rows].T) / 8).astype(FP8),
            "kt": (np.ascontiguousarray(Kt[bidx].T) / 8).astype(FP8),
            "vt": (np.ascontiguousarray(V[bidx].T) / 8).astype(FP8),
            "keep": np.ascontiguousarray(
                (~attn_mask[bidx, rows]).T.astype(BF16)),
            "qres": (np.ascontiguousarray(Q[bidx, rows], f32) + bo_f).astype(BF16),
        }
        m["ident"] = np.eye(P, dtype=BF16)
        m.update(w)
        m.update(b)
        in_maps.append(m)
    return in_maps


def kernel(Q, K, V, attn_mask, Wq, bq, Wk, bk, Wv, bv, Wo, bo, gamma, beta,
           _profile=None):
    from concourse.bass_utils import run_bass_kernel_spmd

    nc = _get_nc()
    in_maps = _make_in_maps(np.asarray(Q, np.float32), np.asarray(K, np.float32),
                            np.asarray(V, np.float32), np.asarray(attn_mask),
                            Wq, bq, Wk, bk, Wv, bv, Wo, bo, gamma, beta)
    kwargs = dict(_profile) if _profile else {}
    res = run_bass_kernel_spmd(nc, in_maps, list(range(NCORES)), **kwargs)
    if _profile is not None:
        _cached["last_results"] = res
    out = np.empty((B, S, D), np.float32)
    for c, m in enumerate(res.results):
        bidx, half = divmod(c, 2)
        out[bidx, half * ROWS:(half + 1) * ROWS] = m["out"]
    return out

